# revision 9
# baseline (speedup 1.0000x reference)
"""AttentionPerLabelWordLevel Trainium2 kernel (8-core SPMD, batch-sharded).

Reference computation (per batch b):
  h = tanh(x @ W.T + b)                      # [T, H]
  logits = h @ C.T                           # [S, L, C]
  m = max_L(logits)                          # [S, 1, C]
  attn = softmax_C(logits - m)               # [S, L, C]
  out[s, c, :] = sum_l attn[s, l, c] * x[s, l, :]   # [S, C, H]

Shapes: B=32, T=2500 (S=100 sentences x L=25 words), H=512, C=50.
Sharding: data-parallel over batch, 4 batches per core.

Per-core strategy (x, W, C are pre-cast to float16 on the host; W and C
are also pre-transposed/padded on the host so the device does no setup
transposes):
  - The 4 batches are treated as one flat space of 400 sentences
    (10000 words), processed in 25 uniform waves of 16 sentences.
  - x is DMA'd once per wave into f16 "packed" tiles [128, 512]
    holding 4 sentences at partition offsets 0/32/64/96.
  - x^T and e^T come from f16 PE transposes into f16 PSUM tiles.
  - Waves are processed in PAIRS with the dense K=128 matmuls of both
    waves emitted back-to-back: the PE HAM clock-gate only counts
    full-array matmuls as activity, so clustering them keeps the PE at
    2.4 GHz and halves the number of cold-clock warmups.
  - Output is stored as f16 (upcast to f32 on the host).
"""

import numpy as np

import concourse.bacc as bacc
import concourse.bass as bass
import concourse.tile as tile
from concourse import mybir
from concourse.bass_utils import run_bass_kernel_spmd
from concourse.masks import make_identity

F32 = mybir.dt.float32
F16 = mybir.dt.float16
AX = mybir.AxisListType
AF = mybir.ActivationFunctionType

N_CORES = 8
B = 32
S = 100          # sentences per batch
L = 25           # words per sentence
C = 50           # classes
H = 512          # hidden
B_LOC = B // N_CORES          # batches per core
S_TOT = B_LOC * S             # flat sentences per core (400)
WAVE_S = 16                   # sentences per wave (4 packed tiles)
N_WAVES = S_TOT // WAVE_S     # 25 uniform waves
G = 4                         # packed tiles per wave
W_COLS = 512                  # padded t-cols per wave

_CACHE = {}
LAST_RESULT = None


def build_nc():
    nc = bacc.Bacc(trn_type="TRN2", target_bir_lowering=False, debug=False,
                   num_swdge_queues=2)
    x_d = nc.declare_dram_parameter("input_tensor", [B_LOC * S * L, H], F16, isOutput=False)
    w_d = nc.declare_dram_parameter("W", [128, 4, H], F16, isOutput=False)
    b_d = nc.declare_dram_parameter("b", [128, 4], F32, isOutput=False)
    c_d = nc.declare_dram_parameter("context_vector", [128, 4, 64], F16, isOutput=False)
    o_d = nc.declare_dram_parameter("out", [B_LOC * S, C, H], F16, isOutput=True)

    q_load = [nc.sync, nc.scalar]
    q_store = [nc.gpsimd, nc.sync]

    with tile.TileContext(nc) as tc:
        with tc.tile_pool(name="sb", bufs=1) as sb, \
             tc.tile_pool(name="consts", bufs=1) as consts, \
             tc.tile_pool(name="ps", bufs=1, space="PSUM") as ps:

            # ---------------- one-time setup (no device transposes) ----------
            ident_f = consts.tile([128, 128], F32)
            make_identity(nc, ident_f)
            ident_h = consts.tile([128, 128], F16)
            nc.vector.tensor_copy(ident_h, ident_f)

            b_sb = consts.tile([128, 4], F32)
            nc.sync.dma_start(out=b_sb, in_=b_d[:, :])

            # W^T: [i-part 128, i_chunk 4, o 512] (f16), host-pretransposed
            w_t = consts.tile([128, 4, 512], F16, name="w_t")
            nc.sync.dma_start(out=w_t, in_=w_d[:, :, :])

            # C^T: [o-part 128, o_chunk 4, c 64] (f16), host-pretransposed
            c_t = consts.tile([128, 4, 64], F16, name="c_t")
            nc.sync.dma_start(out=c_t, in_=c_d[:, :, :])

            # ---------------- per-wave pieces ----------------
            def emit_front(wv):
                s0 = wv * WAVE_S

                # -- load packed f16 x: one DMA per word-row-block jj --
                xp_all = sb.tile([128, 2088], F16, tag="xp", bufs=6,
                                 name=f"xp{wv}")
                for jj in range(4):
                    dvw = xp_all[32 * jj:32 * jj + L, :]
                    dst = bass.AP(tensor=xp_all.tensor, offset=dvw.offset,
                                  ap=[dvw.ap[0], [520, G], [1, 512]])
                    svw = x_d[(s0 + jj) * L:(s0 + jj) * L + 1, :]
                    srcv = bass.AP(tensor=svw.tensor, offset=svw.offset,
                                   ap=[[512, L], [4 * L * 512, G], [1, 512]])
                    q_load[jj % 2].dma_start(out=dst, in_=srcv)

                def xp(g):
                    return xp_all[:, 520 * g:520 * g + 512]

                # -- x^T via f16 PE transposes into full psum banks --
                xt_sb = []
                for half in range(2):
                    pxt = ps.tile([128, 1024], F16, tag="xt", bufs=1,
                                  name=f"pxt{wv}_{half}")
                    for il in range(2):
                        i = 2 * half + il
                        for g in range(G):
                            nc.tensor.transpose(
                                pxt[:, 512 * il + 128 * g:
                                512 * il + 128 * (g + 1)],
                                xp(g)[:, i * 128:(i + 1) * 128],
                                ident_h,
                            )
                    xs = sb.tile([128, 1024], F16, tag="xt_sb", bufs=6,
                                 name=f"xt_sb{wv}_{half}")
                    nc.vector.tensor_copy(xs.bitcast(F32), pxt.bitcast(F32))
                    xt_sb.append(xs)

                return wv, xp_all, xt_sb

            def emit_mm(state):
                """step 1 (h = tanh(Wx+b)) + step 2 (logits) — the dense
                K=128 matmul cluster that keeps the PE HAM-warm."""
                wv, xp_all, xt_sb = state

                def xt_rhs(i):
                    return xt_sb[i // 2][:, 512 * (i % 2):
                                         512 * (i % 2) + W_COLS]

                h = []
                for o in range(4):
                    ph = ps.tile([128, W_COLS], F32, tag="ph", bufs=2,
                                 name=f"ph{wv}_{o}")
                    for i in range(4):
                        nc.tensor.matmul(
                            ph,
                            w_t[:, i, o * 128:(o + 1) * 128],
                            xt_rhs(i),
                            start=(i == 0), stop=(i == 3),
                        )
                    ht = sb.tile([128, 512], F16, tag="h", bufs=8,
                                 name=f"h{wv}_{o}")
                    nc.scalar.activation(
                        out=ht[:, :W_COLS], in_=ph,
                        func=AF.Tanh, bias=b_sb[:, o:o + 1], scale=1.0,
                    )
                    h.append(ht)

                pl = ps.tile([C, W_COLS], F32, tag="soft", bufs=1,
                             name=f"pl{wv}")
                for o in range(4):
                    nc.tensor.matmul(
                        pl, c_t[:, o, :C], h[o][:, :W_COLS],
                        start=(o == 0), stop=(o == 3),
                    )
                return pl

            def emit_soft(state, pl):
                """max over words, e = exp(logits - m) — DVE/ACT only."""
                wv, xp_all, xt_sb = state
                ns = WAVE_S

                m = sb.tile([C, WAVE_S], F32, tag="m", bufs=4,
                            name=f"m{wv}")
                pl_v = bass.AP(tensor=pl.tensor, offset=pl.offset,
                               ap=[pl.ap[0], [32, ns], [1, L]])
                nc.vector.reduce_max(out=m[:, :ns], in_=pl_v, axis=AX.X)

                epre = sb.tile([C, 512], F16, tag="epre", bufs=4,
                               name=f"epre{wv}")
                e_sb = sb.tile([C, 512], F16, tag="e", bufs=4,
                               name=f"e{wv}")
                ep_v = bass.AP(tensor=epre.tensor, offset=epre.offset,
                               ap=[epre.ap[0], [32, ns], [1, L]])
                e_v = bass.AP(tensor=e_sb.tensor, offset=e_sb.offset,
                              ap=[e_sb.ap[0], [32, ns], [1, L]])
                m_v = bass.AP(tensor=m.tensor, offset=m.offset,
                              ap=[m.ap[0], [1, ns], [0, L]])
                nc.vector.tensor_sub(ep_v, pl_v, m_v)
                nc.scalar.activation(out=e_v, in_=ep_v, func=AF.Exp)
                return e_sb

            def emit_out(state, e_sb):
                """e^T transposes, attn normalize, output einsum + store."""
                wv, xp_all, xt_sb = state
                s0 = wv * WAVE_S

                def xp(g):
                    return xp_all[:, 520 * g:520 * g + 512]

                pet = ps.tile([128, 256], F16, tag="soft", bufs=1,
                              name=f"pet{wv}")
                for g in range(G):
                    nc.tensor.transpose(
                        pet[:, 64 * g:64 * g + C],
                        e_sb[:, 128 * g:128 * (g + 1)],
                        ident_h[:C, :C],
                    )
                attn = sb.tile([128, 256], F16, tag="attn", bufs=6,
                               name=f"attn{wv}")
                nc.vector.tensor_copy(attn.bitcast(F32), pet.bitcast(F32))

                # batched normalization: z[g] = 1/sum_c, attn *= z
                # (sum and multiply on GpSimd to unload DVE)
                att_v = bass.AP(tensor=attn.tensor, offset=attn.offset,
                                ap=[attn.ap[0], [64, G], [1, C]])
                z = sb.tile([128, 4], F32, tag="z", bufs=4,
                            name=f"z{wv}")
                nc.vector.reduce_sum(out=z[:, :G], in_=att_v, axis=AX.X)
                nc.vector.reciprocal(out=z[:, :G], in_=z[:, :G])
                z_v = bass.AP(tensor=z.tensor, offset=z.offset,
                              ap=[z.ap[0], [1, G], [0, C]])
                nc.gpsimd.tensor_mul(att_v, att_v, z_v)

                # out[c, o] per sentence; 4xK 2xM packed f16
                si = 0
                for pi in range(2):
                    osb = sb.tile([128, 2088], F16, tag="osb", bufs=6,
                                  name=f"osb{wv}_{pi}")
                    for jj in range(4):
                        po = ps.tile([128, 512], F32, tag=f"po{jj % 2}",
                                 bufs=2, name=f"po{wv}_{pi}_{jj}")
                        for gl in range(2):
                            g = 2 * pi + gl
                            nc.tensor.matmul(
                                po[64 * gl:64 * gl + C, :],
                                attn[32 * jj:32 * jj + L,
                                 64 * g:64 * g + C],
                                xp(g)[32 * jj:32 * jj + L, :],
                                start=True, stop=True,
                                tile_position=(32 * jj, 64 * gl),
                            )
                        # drain each po with one DVE half + one ACT half so
                        # the bank frees at twice the single-engine rate
                        ncols = 64 + C
                        nc.vector.tensor_copy(
                            osb[:ncols, 520 * jj:520 * jj + 256],
                            po[:ncols, :256])
                        nc.scalar.copy(
                            osb[:ncols, 520 * jj + 256:520 * jj + 512],
                            po[:ncols, 256:])
                    for gl in range(2):
                        sbase = s0 + 8 * pi + 4 * gl
                        ovw = osb[64 * gl:64 * gl + C, :]
                        srcv = bass.AP(tensor=osb.tensor, offset=ovw.offset,
                                   ap=[ovw.ap[0], [520, 4], [1, 512]])
                        dvw = o_d[sbase:sbase + 1]
                        dst = bass.AP(tensor=dvw.tensor, offset=dvw.offset,
                                  ap=[[512, C], [C * 512, 4], [1, 512]])
                        q_store[si % 2].dma_start(out=dst, in_=srcv)
                        si += 1

            # ---------------- skewed pipeline ----------------
            # Per wave w the emission order is:
            #   front(w+2) | mm(w) | soft(w) | out(w-1)
            # so the non-HAM-counting PE ops (transposes, out einsum) are
            # interleaved with the dense K=128 matmuls of neighboring waves
            # and the PSUM drains of out(w-1) overlap step1(w)/step1(w+1).
            fronts = {0: emit_front(0), 1: emit_front(1)}
            prev = None
            for w in range(N_WAVES):
                if w + 2 < N_WAVES:
                    fronts[w + 2] = emit_front(w + 2)
                st = fronts.pop(w)
                pl = emit_mm(st)
                e = emit_soft(st, pl)
                if prev is not None:
                    emit_out(*prev)
                prev = (st, e)
            emit_out(*prev)

    nc.compile()
    return nc


def kernel(**inputs):
    global LAST_RESULT
    if "nc" not in _CACHE:
        _CACHE["nc"] = build_nc()
    nc = _CACHE["nc"]

    x = np.asarray(inputs["input_tensor"], dtype=np.float32).astype(np.float16)
    w = np.asarray(inputs["W"], dtype=np.float32)
    bb = np.asarray(inputs["b"], dtype=np.float32)
    cv = np.asarray(inputs["context_vector"], dtype=np.float32)

    # W^T packed as [i-part 128, i_chunk 4, o 512]:  wt[p, i, o] = W[o, 128i+p]
    wt = np.ascontiguousarray(
        w.T.reshape(4, 128, H).transpose(1, 0, 2)).astype(np.float16)
    # C^T packed as [o-part 128, o_chunk 4, c 64]:  ct[p, o, c] = C[c, 128o+p]
    ct = np.zeros((128, 4, 64), dtype=np.float16)
    ct[:, :, :C] = cv.T.reshape(4, 128, C).transpose(1, 0, 2)
    # bias packed as [p 128, o_chunk 4]
    b2 = np.ascontiguousarray(bb.reshape(4, 128).T)

    in_maps = [
        {
            "input_tensor": np.ascontiguousarray(
                x[ci * B_LOC:(ci + 1) * B_LOC].reshape(B_LOC * S * L, H)),
            "W": wt,
            "b": b2,
            "context_vector": ct,
        }
        for ci in range(N_CORES)
    ]
    res = run_bass_kernel_spmd(nc, in_maps, core_ids=list(range(N_CORES)))
    LAST_RESULT = res
    out = np.empty((B, S, C, H), dtype=np.float32)
    for ci in range(N_CORES):
        out[ci * B_LOC:(ci + 1) * B_LOC] = (
            res.results[ci]["out"].astype(np.float32).reshape(B_LOC, S, C, H))
    return out
